# revision 27
# baseline (speedup 1.0000x reference)
"""Multi-head attention (B=8, S=1500, E=1024, H=16, D=64) on 8 trn2 NeuronCores.

Sharding: pure data-parallel over batch — core b computes batch element b
end-to-end (no collectives). Host pre-transposes x and the weights so every
device-side matmul has its contraction dim on the SBUF partition axis, and
folds the 1/sqrt(D) scale into Wq/bq and the V-bias into the output bias
(bo_eff = bo + Wo @ bv), so the device kernel never touches bv.

Device pipeline per core (all f32 storage, matmuls run as float32r):
  QT = (Wq/8)^T-proj of x^T   [1024, 1500]  (f-on-partition; bias bq/8 per-partition)
  KT = Wk^T-proj              [1024, 1500]
  V_aug = x @ Wv^T with a ones-column appended per head  [1500, 16*65]
  per (i-chunk, head): scoresT[j, i] via matmul(lhsT=KT_h, rhs=QT_h);
    exp on ACT (no max-subtraction: |scores| <~ 30, safely inside f32);
    causal masking structurally (affine_select zero-fill on diagonal blocks)
    or via an additive mask tensor (general path);
  out^T + rowsums in ONE matmul: lhsT=[V_h | 1] (65 cols), rhs=attnT;
  normalize: recip of the sums row, rank-1 matmul broadcast across 64
    partitions, multiply on eviction -> AO^T;
  yT = Wo^T-proj of AO^T + bo_eff  -> DRAM [1024, 1500], host transposes back.
"""

import sys
import numpy as np
import ml_dtypes

for _p in ("/opt/trn_rl_repo",):
    if _p not in sys.path:
        sys.path.append(_p)

import concourse.bass as bass
import concourse.mybir as mybir
import concourse.tile as tile
from concourse import bacc
from concourse.bass_utils import run_bass_kernel_spmd

F32 = mybir.dt.float32

B, S, E, H, D = 8, 1500, 1024, 16, 64
P = 128
NEG = -1e9


def _chunks(total, step):
    return [(c0, min(step, total - c0)) for c0 in range(0, total, step)]


def _wslices(dram_ap, col0, cols):
    """[E, E] weight -> [P, E//P, cols] AP for a column slice (k on partition)."""
    return dram_ap.rearrange("(kt p) f -> p kt f", p=P)[:, :, col0:col0 + cols]


def build(causal: bool, mm_dt=mybir.dt.bfloat16):
    KT_N = E // P            # k-tiles over the embedding dim
    FT_N = E // P            # f-tiles
    R_CH = _chunks(S, 512)   # i/r chunks
    JB = _chunks(S, P)       # j blocks
    H_LOC = E // D
    nc = bacc.Bacc("TRN2", target_bir_lowering=False, debug=False, num_devices=8)
    MD = mm_dt  # dtype for every matmul operand chain
    NRM = mybir.dt.float32r if MD == mybir.dt.bfloat16 else MD

    xT = nc.dram_tensor("xT", [E, S], MD, kind="ExternalInput")
    wqT = nc.dram_tensor("wqT", [E, E], MD, kind="ExternalInput")
    wkT = nc.dram_tensor("wkT", [E, E], MD, kind="ExternalInput")
    wvT = nc.dram_tensor("wvT", [E, E], MD, kind="ExternalInput")
    woT = nc.dram_tensor("woT", [E, E], MD, kind="ExternalInput")
    bq = nc.dram_tensor("bq", [E], F32, kind="ExternalInput")
    bo = nc.dram_tensor("bo", [E], F32, kind="ExternalInput")
    maskT = None
    if not causal:
        maskT = nc.dram_tensor("maskT", [S, S], F32, kind="ExternalInput")
    yT = nc.dram_tensor("yT", [E, S], F32, kind="ExternalOutput")

    def mm(ap):
        return ap

    nc._allow_low_precision_reason = "low-precision matmul operand chain"
    with tile.TileContext(nc) as tc:
        with (
            tc.tile_pool(name="persist", bufs=1) as pers,
            tc.tile_pool(name="wqkp", bufs=2) as wqkp,
            tc.tile_pool(name="wvp", bufs=1) as wvp,
            tc.tile_pool(name="wop", bufs=1) as wop,
            tc.tile_pool(name="attn", bufs=3) as apool,
            tc.tile_pool(name="small", bufs=3) as spool,
            tc.tile_pool(name="evp", bufs=3) as evp,
            tc.tile_pool(name="psP", bufs=1, space="PSUM") as psP,
            tc.tile_pool(name="psS", bufs=1, space="PSUM") as psS,
            tc.tile_pool(name="psO", bufs=1, space="PSUM") as psO,
            tc.tile_pool(name="psY", bufs=1, space="PSUM") as psY,
        ):
            ones64 = pers.tile([1, D], NRM, name="ones64")
            nc.vector.memset(ones64[:].bitcast(F32), 1.0)
            bq_sb = pers.tile([P, FT_N], F32, name="bq_sb")
            nc.sync.dma_start(out=bq_sb[:], in_=bq.ap().rearrange("(t p) -> p t", p=P))
            bo_sb = pers.tile([P, FT_N], F32, name="bo_sb")
            nc.sync.dma_start(out=bo_sb[:], in_=bo.ap().rearrange("(t p) -> p t", p=P))

            # upper-triangular (incl diag) 0/1 mask for diagonal attn blocks
            tri32 = pers.tile([P, P], F32, name="tri32")
            nc.gpsimd.memset(tri32[:], 1.0)
            nc.gpsimd.affine_select(
                out=tri32[:], in_=tri32[:],
                pattern=[[1, P]], compare_op=mybir.AluOpType.is_ge,
                fill=0.0, base=0, channel_multiplier=-1,
            )
            tri = pers.tile([P, P], MD, name="tri")
            nc.vector.tensor_copy(out=tri[:], in_=tri32[:])

            XT = [pers.tile([P, S], MD, name=f"xt{kt}") for kt in range(KT_N)]
            QT = [pers.tile([P, S], MD, name=f"qt{ft}") for ft in range(FT_N)]
            KTs = [pers.tile([P, S], MD, name=f"kt{ft}") for ft in range(FT_N)]
            VA = [pers.tile([P, H_LOC * (D + 1)], MD, name=f"va{rt}")
                  for rt in range(len(JB))]
            AOT = [pers.tile([P, S], MD, name=f"aot{ft}") for ft in range(FT_N)]

            # ---- V projection (natural layout, ones column appended) ----
            hpc = 512 // D  # heads per 512-wide f chunk
            fchunks = _chunks(E, 512)
            wv_tiles = [wvp.tile([P, KT_N, 512], MD, name=f"wv{fc}", tag=f"wv{fc}")
                        for fc in range(len(fchunks))]
            # interleave wv-slice and xT-tile loads so the first V matmuls
            # can issue as soon as (wv[:,0,:], xT[0]) land
            for kt in range(KT_N):
                for fc, (f0, fw) in enumerate(fchunks):
                    nc.sync.dma_start(
                        out=wv_tiles[fc][:, kt, :fw],
                        in_=_wslices(wvT.ap(), f0, fw)[:, kt, :])
                nc.sync.dma_start(out=XT[kt][:],
                                  in_=xT[kt * P:(kt + 1) * P, :])

            def emit_v(rts):
                for rt in rts:
                    if rt >= len(JB):
                        continue
                    r0, rsz = JB[rt]
                    for fc, (f0, fw) in enumerate(fchunks):
                        wt = wv_tiles[fc]
                        ps = psP.tile([P, 512], F32, name="pv", tag="pp", bufs=2)
                        for kt in range(KT_N):
                            nc.tensor.matmul(
                                ps[:rsz, :fw],
                                mm(XT[kt][:, r0:r0 + rsz]),
                                mm(wt[:, kt, :fw]),
                                start=(kt == 0), stop=(kt == KT_N - 1),
                            )
                        dst = VA[rt][:].rearrange("p (h c) -> p h c", c=D + 1)
                        nc.vector.tensor_copy(
                            out=dst[:rsz, fc * hpc:fc * hpc + fw // D, 0:D],
                            in_=ps[:rsz, :fw].rearrange("p (h d) -> p h d", d=D),
                        )
                    va3 = VA[rt][:].rearrange("p (h c) -> p h c", c=D + 1)
                    if MD == mybir.dt.float32r:
                        nc.gpsimd.memset(va3[:rsz, :, D:D + 1].bitcast(F32), 1.0)
                    else:
                        nc.gpsimd.memset(va3[:rsz, :, D:D + 1], 1.0)

            def proj_qk_gen(ft):
                for which, wdram, dst in (("q", wqT, QT), ("k", wkT, KTs)):
                    wt = wqkp.tile([P, KT_N, P], MD, name="wqk", tag="wqk")
                    nc.sync.dma_start(out=wt[:], in_=_wslices(wdram.ap(), ft * P, P))
                    for rc, (c0, cw) in enumerate(R_CH):
                        ps = psP.tile([P, 512], F32, name="pp", tag="pp", bufs=2)
                        for kt in range(KT_N):
                            nc.tensor.matmul(
                                ps[:, :cw],
                                mm(wt[:, kt, :]),
                                mm(XT[kt][:, c0:c0 + cw]),
                                start=(kt == 0), stop=(kt == KT_N - 1),
                            )
                        if which == "q":
                            nc.vector.tensor_scalar(
                                out=dst[ft][:, c0:c0 + cw], in0=ps[:, :cw],
                                scalar1=bq_sb[:, ft:ft + 1], scalar2=None,
                                op0=mybir.AluOpType.add,
                            )
                        else:
                            nc.vector.tensor_copy(
                                out=dst[ft][:, c0:c0 + cw], in_=ps[:, :cw])
                        yield

            def proj_qk(ft):
                for _ in proj_qk_gen(ft):
                    pass

            def attn_ft(ic, ft, mtiles, filler=None):
                c0, cw = R_CH[ic]
                nblk = (min(len(JB), (c0 + cw + P - 1) // P)
                        if causal else len(JB))
                pso = [psO.tile([D + 1, 512], F32, name=f"po{half}",
                                tag="po", bufs=2)
                       for half in range(2)]
                # diagonal-containing blocks first so the chunk-end attnV
                # gates on a short (non-masked) exp chain
                if causal:
                    cut = max(0, nblk - (cw + P - 1) // P)
                    order = list(range(cut, nblk)) + list(range(cut))
                else:
                    order = list(range(nblk))
                for n_i, jb in enumerate(order):
                    j0, jsz = JB[jb]
                    vo = max(0, j0 - c0) if causal else 0
                    pss, ats = [], []
                    for half in range(2):
                        d0 = D * half
                        ps = psS.tile([P, 512], F32, name=f"ps{half}",
                                      tag="ps", bufs=3)
                        # adjacent half-pair at row groups (0,0)/(64,0):
                        # runs concurrently in the PE array
                        nc.tensor.matmul(
                            ps[:jsz, vo:cw],
                            mm(KTs[ft][d0:d0 + D, j0:j0 + jsz]),
                            mm(QT[ft][d0:d0 + D, c0 + vo:c0 + cw]),
                            start=True, stop=True,
                            tile_position=(d0, 0),
                        )
                        pss.append(ps)
                    for half in range(2):
                        ps = pss[half]
                        if not causal:
                            nc.vector.tensor_tensor(
                                out=ps[:jsz, :cw], in0=ps[:jsz, :cw],
                                in1=mtiles[jb][:jsz, :cw],
                                op=mybir.AluOpType.add,
                            )
                        at = apool.tile([P, 512], MD, name=f"at{half}")
                        nc.scalar.activation(
                            out=at[:jsz, vo:cw], in_=ps[:jsz, vo:cw],
                            func=mybir.ActivationFunctionType.Exp,
                        )
                        if causal and j0 >= c0:
                            # zero attn where j > i on the diagonal square
                            nc.vector.tensor_tensor(
                                out=at[:jsz, vo:vo + jsz],
                                in0=at[:jsz, vo:vo + jsz],
                                in1=tri[:jsz, :jsz],
                                op=mybir.AluOpType.mult,
                            )
                        ats.append(at)
                    va3 = VA[jb][:].rearrange("p (h c) -> p h c", c=D + 1)
                    for half in range(2):
                        nc.tensor.matmul(
                            pso[half][:, vo:cw],
                            mm(va3[:jsz, 2 * ft + half, :]),
                            mm(ats[half][:jsz, vo:cw]),
                            start=(n_i == 0), stop=(n_i == nblk - 1),
                        )
                    if filler is not None and n_i % 4 == 3:
                        filler()
                ssums = []
                for half in range(2):
                    ssum = spool.tile([1, 512], NRM, name=f"ssum{half}",
                                      tag="ssum")
                    nc.vector.tensor_copy(
                        out=ssum[:, :cw], in_=pso[half][D:D + 1, :cw])
                    ssums.append(ssum)
                for half in range(2):
                    d0 = D * half
                    psb = psY.tile([D, 512], F32, name="psb", tag="pyb", bufs=1)
                    nc.tensor.matmul(
                        psb[:, :cw], mm(ones64[:, :]), mm(ssums[half][:, :cw]),
                        start=True, stop=True,
                    )
                    rb = spool.tile([D, 512], F32, name="rb")
                    nc.vector.reciprocal_approx_fast(
                        out=rb[:, :cw], in_=psb[:, :cw])
                    nc.vector.tensor_tensor(
                        out=AOT[ft][d0:d0 + D, c0:c0 + cw],
                        in0=pso[half][0:D, :cw], in1=rb[:, :cw],
                        op=mybir.AluOpType.mult,
                    )

            def emit_yt(ot, rc, wo_t):
                c0, cw = R_CH[rc]
                psy = psP.tile([P, 512], F32, name="py", tag="pp", bufs=2)
                for ft in range(FT_N):
                    nc.tensor.matmul(
                        psy[:, :cw],
                        mm(wo_t[:, ft, :]),
                        mm(AOT[ft][:, c0:c0 + cw]),
                        start=(ft == 0), stop=(ft == FT_N - 1),
                    )
                yt = evp.tile([P, 512], F32, name="yt", tag="yt")
                nc.vector.tensor_scalar(
                    out=yt[:, :cw], in0=psy[:, :cw],
                    scalar1=bo_sb[:, ot:ot + 1], scalar2=None,
                    op0=mybir.AluOpType.add,
                )
                nc.sync.dma_start(
                    out=yT[ot * P:(ot + 1) * P, c0:c0 + cw], in_=yt[:, :cw])

            if causal:
                nb0 = min(len(JB), (R_CH[0][0] + R_CH[0][1] + P - 1) // P)
                emit_v(range(nb0))
                proj_qk(0)
                nbp = nb0
                for ft in range(FT_N):
                    gen = proj_qk_gen(ft + 1) if ft + 1 < FT_N else None

                    def pump():
                        if gen is not None:
                            next(gen, None)

                    for ic in range(len(R_CH)):
                        attn_ft(ic, ft, None, filler=pump)
                        if ft == 0 and ic + 1 < len(R_CH):
                            c0n, cwn = R_CH[ic + 1]
                            nbn = min(len(JB), (c0n + cwn + P - 1) // P)
                            emit_v(range(nbp, nbn))
                            nbp = nbn
                    if gen is not None:
                        for _ in gen:
                            pass
                for ot in range(FT_N):
                    wt = wop.tile([P, KT_N, P], MD, name=f"wo{ot}", tag="wo",
                                  bufs=2)
                    nc.sync.dma_start(out=wt[:], in_=_wslices(woT.ap(), ot * P, P))
                    for rc in range(len(R_CH)):
                        emit_yt(ot, rc, wt)
            else:
                emit_v(range(len(JB)))
                for ft in range(FT_N):
                    proj_qk(ft)
                with tc.tile_pool(name="maskp", bufs=1) as mpool:
                    for ic, (c0, cw) in enumerate(R_CH):
                        mtiles = []
                        for jb, (j0, jsz) in enumerate(JB):
                            mt = mpool.tile([P, 512], F32, name=f"m{jb}")
                            nc.sync.dma_start(
                                out=mt[:jsz, :cw],
                                in_=maskT[j0:j0 + jsz, c0:c0 + cw])
                            mtiles.append(mt)
                        for ft in range(FT_N):
                            attn_ft(ic, ft, mtiles)
                for ot in range(FT_N):
                    wt = wop.tile([P, KT_N, P], MD, name=f"wo{ot}", tag="wo",
                                  bufs=2)
                    nc.sync.dma_start(out=wt[:], in_=_wslices(woT.ap(), ot * P, P))
                    for rc in range(len(R_CH)):
                        emit_yt(ot, rc, wt)

    nc.compile()
    return nc


_CACHE: dict = {}


def _get_nc(causal: bool):
    if causal not in _CACHE:
        _CACHE[causal] = build(causal)
    return _CACHE[causal]


def _is_causal(mask: np.ndarray) -> bool:
    if mask.shape != (S, S):
        return False
    expect = np.where(np.tril(np.ones((S, S), dtype=bool)), np.float32(0.0),
                      np.float32(NEG))
    return bool(np.array_equal(mask, expect))


MM_NP = ml_dtypes.bfloat16  # numpy dtype matching build()'s default mm_dt


def prep_inputs(x, mask, Wq, bq, Wk, Wv, bv, Wo, bo):
    """Host-side preprocessing shared by kernel() and the bench harness."""
    scale = np.float32(1.0 / np.sqrt(D))
    xT = np.ascontiguousarray(np.transpose(x, (0, 2, 1)).astype(np.float32)).astype(MM_NP)
    common = {
        "wqT": np.ascontiguousarray((Wq.astype(np.float32) * scale).T).astype(MM_NP),
        "wkT": np.ascontiguousarray(Wk.astype(np.float32).T).astype(MM_NP),
        "wvT": np.ascontiguousarray(Wv.astype(np.float32).T).astype(MM_NP),
        "woT": np.ascontiguousarray(Wo.astype(np.float32).T).astype(MM_NP),
        "bq": (bq.astype(np.float32) * scale),
        "bo": (bo.astype(np.float32) + Wo.astype(np.float32) @ bv.astype(np.float32)),
    }
    causal = _is_causal(np.asarray(mask))
    if not causal:
        common["maskT"] = np.ascontiguousarray(np.asarray(mask, np.float32).T)
    in_maps = [dict(common, xT=xT[b]) for b in range(B)]
    return causal, in_maps


def kernel(x, mask, Wq, bq, Wk, Wv, bv, Wo, bo):
    causal, in_maps = prep_inputs(x, mask, Wq, bq, Wk, Wv, bv, Wo, bo)
    nc = _get_nc(causal)
    res = run_bass_kernel_spmd(nc, in_maps, list(range(B))).results
    out = np.stack([res[b]["yT"].T for b in range(B)])
    return np.ascontiguousarray(out.astype(np.float32))


# revision 28
# speedup vs baseline: 1.0323x; 1.0323x over previous
"""Multi-head attention (B=8, S=1500, E=1024, H=16, D=64) on 8 trn2 NeuronCores.

Sharding: pure data-parallel over batch — core b computes batch element b
end-to-end (no collectives). Host pre-transposes x and the weights so every
device-side matmul has its contraction dim on the SBUF partition axis, and
folds the 1/sqrt(D) scale into Wq/bq and the V-bias into the output bias
(bo_eff = bo + Wo @ bv), so the device kernel never touches bv.

Device pipeline per core (all f32 storage, matmuls run as float32r):
  QT = (Wq/8)^T-proj of x^T   [1024, 1500]  (f-on-partition; bias bq/8 per-partition)
  KT = Wk^T-proj              [1024, 1500]
  V_aug = x @ Wv^T with a ones-column appended per head  [1500, 16*65]
  per (i-chunk, head): scoresT[j, i] via matmul(lhsT=KT_h, rhs=QT_h);
    exp on ACT (no max-subtraction: |scores| <~ 30, safely inside f32);
    causal masking structurally (affine_select zero-fill on diagonal blocks)
    or via an additive mask tensor (general path);
  out^T + rowsums in ONE matmul: lhsT=[V_h | 1] (65 cols), rhs=attnT;
  normalize: recip of the sums row, rank-1 matmul broadcast across 64
    partitions, multiply on eviction -> AO^T;
  yT = Wo^T-proj of AO^T + bo_eff  -> DRAM [1024, 1500], host transposes back.
"""

import sys
import numpy as np
import ml_dtypes

for _p in ("/opt/trn_rl_repo",):
    if _p not in sys.path:
        sys.path.append(_p)

import concourse.bass as bass
import concourse.mybir as mybir
import concourse.tile as tile
from concourse import bacc
from concourse.bass_utils import run_bass_kernel_spmd

F32 = mybir.dt.float32

B, S, E, H, D = 8, 1500, 1024, 16, 64
P = 128
NEG = -1e9


def _chunks(total, step):
    return [(c0, min(step, total - c0)) for c0 in range(0, total, step)]


def _wslices(dram_ap, col0, cols):
    """[E, E] weight -> [P, E//P, cols] AP for a column slice (k on partition)."""
    return dram_ap.rearrange("(kt p) f -> p kt f", p=P)[:, :, col0:col0 + cols]


def build(causal: bool, mm_dt=mybir.dt.bfloat16):
    KT_N = E // P            # k-tiles over the embedding dim
    FT_N = E // P            # f-tiles
    R_CH = _chunks(S, 512)   # i/r chunks
    JB = _chunks(S, P)       # j blocks
    H_LOC = E // D
    nc = bacc.Bacc("TRN2", target_bir_lowering=False, debug=False, num_devices=8)
    MD = mm_dt  # dtype for every matmul operand chain
    NRM = mybir.dt.float32r if MD == mybir.dt.bfloat16 else MD

    xT = nc.dram_tensor("xT", [E, S], MD, kind="ExternalInput")
    wqT = nc.dram_tensor("wqT", [E, E], MD, kind="ExternalInput")
    wkT = nc.dram_tensor("wkT", [E, E], MD, kind="ExternalInput")
    wvT = nc.dram_tensor("wvT", [E, E], MD, kind="ExternalInput")
    woT = nc.dram_tensor("woT", [E, E], MD, kind="ExternalInput")
    bq = nc.dram_tensor("bq", [E], F32, kind="ExternalInput")
    bo = nc.dram_tensor("bo", [E], F32, kind="ExternalInput")
    maskT = None
    if not causal:
        maskT = nc.dram_tensor("maskT", [S, S], F32, kind="ExternalInput")
    yT = nc.dram_tensor("yT", [E, S], F32, kind="ExternalOutput")

    def mm(ap):
        return ap

    nc._allow_low_precision_reason = "low-precision matmul operand chain"
    with tile.TileContext(nc) as tc:
        with (
            tc.tile_pool(name="persist", bufs=1) as pers,
            tc.tile_pool(name="wqkp", bufs=2) as wqkp,
            tc.tile_pool(name="wvp", bufs=1) as wvp,
            tc.tile_pool(name="wop", bufs=1) as wop,
            tc.tile_pool(name="attn", bufs=3) as apool,
            tc.tile_pool(name="small", bufs=3) as spool,
            tc.tile_pool(name="evp", bufs=3) as evp,
            tc.tile_pool(name="psP", bufs=1, space="PSUM") as psP,
            tc.tile_pool(name="psS", bufs=1, space="PSUM") as psS,
            tc.tile_pool(name="psO", bufs=1, space="PSUM") as psO,
            tc.tile_pool(name="psY", bufs=1, space="PSUM") as psY,
        ):
            ones64 = pers.tile([1, D], NRM, name="ones64")
            nc.vector.memset(ones64[:].bitcast(F32), 1.0)
            bq_sb = pers.tile([P, FT_N], F32, name="bq_sb")
            nc.sync.dma_start(out=bq_sb[:], in_=bq.ap().rearrange("(t p) -> p t", p=P))
            bo_sb = pers.tile([P, FT_N], F32, name="bo_sb")
            nc.sync.dma_start(out=bo_sb[:], in_=bo.ap().rearrange("(t p) -> p t", p=P))

            # upper-triangular (incl diag) 0/1 mask for diagonal attn blocks
            tri32 = pers.tile([P, P], F32, name="tri32")
            nc.gpsimd.memset(tri32[:], 1.0)
            nc.gpsimd.affine_select(
                out=tri32[:], in_=tri32[:],
                pattern=[[1, P]], compare_op=mybir.AluOpType.is_ge,
                fill=0.0, base=0, channel_multiplier=-1,
            )
            tri = pers.tile([P, P], MD, name="tri")
            nc.vector.tensor_copy(out=tri[:], in_=tri32[:])

            XT = [pers.tile([P, S], MD, name=f"xt{kt}") for kt in range(KT_N)]
            QT = [pers.tile([P, S], MD, name=f"qt{ft}") for ft in range(FT_N)]
            KTs = [pers.tile([P, S], MD, name=f"kt{ft}") for ft in range(FT_N)]
            VA = [pers.tile([P, H_LOC * (D + 1)], MD, name=f"va{rt}")
                  for rt in range(len(JB))]
            AOT = [pers.tile([P, S], MD, name=f"aot{ft}") for ft in range(FT_N)]

            # ---- V projection (natural layout, ones column appended) ----
            hpc = 512 // D  # heads per 512-wide f chunk
            fchunks = _chunks(E, 512)
            wv_tiles = [wvp.tile([P, KT_N, 512], MD, name=f"wv{fc}", tag=f"wv{fc}")
                        for fc in range(len(fchunks))]
            # interleave wv-slice and xT-tile loads so the first V matmuls
            # can issue as soon as (wv[:,0,:], xT[0]) land
            for kt in range(KT_N):
                for fc, (f0, fw) in enumerate(fchunks):
                    nc.sync.dma_start(
                        out=wv_tiles[fc][:, kt, :fw],
                        in_=_wslices(wvT.ap(), f0, fw)[:, kt, :])
                nc.sync.dma_start(out=XT[kt][:],
                                  in_=xT[kt * P:(kt + 1) * P, :])

            def emit_v(rts):
                for rt in rts:
                    if rt >= len(JB):
                        continue
                    r0, rsz = JB[rt]
                    for fc, (f0, fw) in enumerate(fchunks):
                        wt = wv_tiles[fc]
                        ps = psP.tile([P, 512], F32, name="pv", tag="pp", bufs=2)
                        for kt in range(KT_N):
                            nc.tensor.matmul(
                                ps[:rsz, :fw],
                                mm(XT[kt][:, r0:r0 + rsz]),
                                mm(wt[:, kt, :fw]),
                                start=(kt == 0), stop=(kt == KT_N - 1),
                            )
                        dst = VA[rt][:].rearrange("p (h c) -> p h c", c=D + 1)
                        nc.vector.tensor_copy(
                            out=dst[:rsz, fc * hpc:fc * hpc + fw // D, 0:D],
                            in_=ps[:rsz, :fw].rearrange("p (h d) -> p h d", d=D),
                        )
                    va3 = VA[rt][:].rearrange("p (h c) -> p h c", c=D + 1)
                    if MD == mybir.dt.float32r:
                        nc.gpsimd.memset(va3[:rsz, :, D:D + 1].bitcast(F32), 1.0)
                    else:
                        nc.gpsimd.memset(va3[:rsz, :, D:D + 1], 1.0)

            def proj_qk_gen(ft):
                for which, wdram, dst in (("q", wqT, QT), ("k", wkT, KTs)):
                    wt = wqkp.tile([P, KT_N, P], MD, name="wqk", tag="wqk")
                    nc.sync.dma_start(out=wt[:], in_=_wslices(wdram.ap(), ft * P, P))
                    for rc, (c0, cw) in enumerate(R_CH):
                        ps = psP.tile([P, 512], F32, name="pp", tag="pp", bufs=2)
                        for kt in range(KT_N):
                            nc.tensor.matmul(
                                ps[:, :cw],
                                mm(wt[:, kt, :]),
                                mm(XT[kt][:, c0:c0 + cw]),
                                start=(kt == 0), stop=(kt == KT_N - 1),
                            )
                        if which == "q":
                            nc.vector.tensor_scalar(
                                out=dst[ft][:, c0:c0 + cw], in0=ps[:, :cw],
                                scalar1=bq_sb[:, ft:ft + 1], scalar2=None,
                                op0=mybir.AluOpType.add,
                            )
                        else:
                            nc.vector.tensor_copy(
                                out=dst[ft][:, c0:c0 + cw], in_=ps[:, :cw])
                        yield

            def proj_qk(ft):
                for _ in proj_qk_gen(ft):
                    pass

            def attn_ft(ic, ft, mtiles, filler=None):
                c0, cw = R_CH[ic]
                nblk = (min(len(JB), (c0 + cw + P - 1) // P)
                        if causal else len(JB))
                pso = [psO.tile([D + 1, 512], F32, name=f"po{half}",
                                tag="po", bufs=2)
                       for half in range(2)]
                # diagonal-containing blocks first so the chunk-end attnV
                # gates on a short (non-masked) exp chain
                order = list(range(nblk))
                for n_i, jb in enumerate(order):
                    j0, jsz = JB[jb]
                    vo = max(0, j0 - c0) if causal else 0
                    pss, ats = [], []
                    for half in range(2):
                        d0 = D * half
                        ps = psS.tile([P, 512], F32, name=f"ps{half}",
                                      tag="ps", bufs=3)
                        # adjacent half-pair at row groups (0,0)/(64,0):
                        # runs concurrently in the PE array
                        nc.tensor.matmul(
                            ps[:jsz, vo:cw],
                            mm(KTs[ft][d0:d0 + D, j0:j0 + jsz]),
                            mm(QT[ft][d0:d0 + D, c0 + vo:c0 + cw]),
                            start=True, stop=True,
                            tile_position=(d0, 0),
                        )
                        pss.append(ps)
                    for half in range(2):
                        ps = pss[half]
                        if not causal:
                            nc.vector.tensor_tensor(
                                out=ps[:jsz, :cw], in0=ps[:jsz, :cw],
                                in1=mtiles[jb][:jsz, :cw],
                                op=mybir.AluOpType.add,
                            )
                        at = apool.tile([P, 512], MD, name=f"at{half}")
                        nc.scalar.activation(
                            out=at[:jsz, vo:cw], in_=ps[:jsz, vo:cw],
                            func=mybir.ActivationFunctionType.Exp,
                        )
                        if causal and j0 >= c0:
                            # zero attn where j > i on the diagonal square
                            nc.vector.tensor_tensor(
                                out=at[:jsz, vo:vo + jsz],
                                in0=at[:jsz, vo:vo + jsz],
                                in1=tri[:jsz, :jsz],
                                op=mybir.AluOpType.mult,
                            )
                        ats.append(at)
                    va3 = VA[jb][:].rearrange("p (h c) -> p h c", c=D + 1)
                    for half in range(2):
                        nc.tensor.matmul(
                            pso[half][:, vo:cw],
                            mm(va3[:jsz, 2 * ft + half, :]),
                            mm(ats[half][:jsz, vo:cw]),
                            start=(n_i == 0), stop=(n_i == nblk - 1),
                        )
                    if filler is not None and n_i % 4 == 3:
                        filler()
                ssums = []
                for half in range(2):
                    ssum = spool.tile([1, 512], NRM, name=f"ssum{half}",
                                      tag="ssum")
                    nc.vector.tensor_copy(
                        out=ssum[:, :cw], in_=pso[half][D:D + 1, :cw])
                    ssums.append(ssum)
                for half in range(2):
                    d0 = D * half
                    psb = psY.tile([D, 512], F32, name="psb", tag="pyb", bufs=1)
                    nc.tensor.matmul(
                        psb[:, :cw], mm(ones64[:, :]), mm(ssums[half][:, :cw]),
                        start=True, stop=True,
                    )
                    rb = spool.tile([D, 512], F32, name="rb")
                    nc.vector.reciprocal_approx_fast(
                        out=rb[:, :cw], in_=psb[:, :cw])
                    nc.vector.tensor_tensor(
                        out=AOT[ft][d0:d0 + D, c0:c0 + cw],
                        in0=pso[half][0:D, :cw], in1=rb[:, :cw],
                        op=mybir.AluOpType.mult,
                    )

            def emit_yt(ot, rc, wo_t):
                c0, cw = R_CH[rc]
                psy = psP.tile([P, 512], F32, name="py", tag="pp", bufs=2)
                for ft in range(FT_N):
                    nc.tensor.matmul(
                        psy[:, :cw],
                        mm(wo_t[:, ft, :]),
                        mm(AOT[ft][:, c0:c0 + cw]),
                        start=(ft == 0), stop=(ft == FT_N - 1),
                    )
                yt = evp.tile([P, 512], F32, name="yt", tag="yt")
                nc.vector.tensor_scalar(
                    out=yt[:, :cw], in0=psy[:, :cw],
                    scalar1=bo_sb[:, ot:ot + 1], scalar2=None,
                    op0=mybir.AluOpType.add,
                )
                nc.sync.dma_start(
                    out=yT[ot * P:(ot + 1) * P, c0:c0 + cw], in_=yt[:, :cw])

            if causal:
                nb0 = min(len(JB), (R_CH[0][0] + R_CH[0][1] + P - 1) // P)
                emit_v(range(nb0))
                proj_qk(0)
                nbp = nb0
                for ft in range(FT_N):
                    gen = proj_qk_gen(ft + 1) if ft + 1 < FT_N else None

                    def pump():
                        if gen is not None:
                            next(gen, None)

                    for ic in range(len(R_CH)):
                        attn_ft(ic, ft, None, filler=pump)
                        if ft == 0 and ic + 1 < len(R_CH):
                            c0n, cwn = R_CH[ic + 1]
                            nbn = min(len(JB), (c0n + cwn + P - 1) // P)
                            emit_v(range(nbp, nbn))
                            nbp = nbn
                    if gen is not None:
                        for _ in gen:
                            pass
                for ot in range(FT_N):
                    wt = wop.tile([P, KT_N, P], MD, name=f"wo{ot}", tag="wo",
                                  bufs=2)
                    nc.sync.dma_start(out=wt[:], in_=_wslices(woT.ap(), ot * P, P))
                    for rc in range(len(R_CH)):
                        emit_yt(ot, rc, wt)
            else:
                emit_v(range(len(JB)))
                for ft in range(FT_N):
                    proj_qk(ft)
                with tc.tile_pool(name="maskp", bufs=1) as mpool:
                    for ic, (c0, cw) in enumerate(R_CH):
                        mtiles = []
                        for jb, (j0, jsz) in enumerate(JB):
                            mt = mpool.tile([P, 512], F32, name=f"m{jb}")
                            nc.sync.dma_start(
                                out=mt[:jsz, :cw],
                                in_=maskT[j0:j0 + jsz, c0:c0 + cw])
                            mtiles.append(mt)
                        for ft in range(FT_N):
                            attn_ft(ic, ft, mtiles)
                for ot in range(FT_N):
                    wt = wop.tile([P, KT_N, P], MD, name=f"wo{ot}", tag="wo",
                                  bufs=2)
                    nc.sync.dma_start(out=wt[:], in_=_wslices(woT.ap(), ot * P, P))
                    for rc in range(len(R_CH)):
                        emit_yt(ot, rc, wt)

    nc.compile()
    return nc


_CACHE: dict = {}


def _get_nc(causal: bool):
    if causal not in _CACHE:
        _CACHE[causal] = build(causal)
    return _CACHE[causal]


def _is_causal(mask: np.ndarray) -> bool:
    if mask.shape != (S, S):
        return False
    expect = np.where(np.tril(np.ones((S, S), dtype=bool)), np.float32(0.0),
                      np.float32(NEG))
    return bool(np.array_equal(mask, expect))


MM_NP = ml_dtypes.bfloat16  # numpy dtype matching build()'s default mm_dt


def prep_inputs(x, mask, Wq, bq, Wk, Wv, bv, Wo, bo):
    """Host-side preprocessing shared by kernel() and the bench harness."""
    scale = np.float32(1.0 / np.sqrt(D))
    xT = np.ascontiguousarray(np.transpose(x, (0, 2, 1)).astype(np.float32)).astype(MM_NP)
    common = {
        "wqT": np.ascontiguousarray((Wq.astype(np.float32) * scale).T).astype(MM_NP),
        "wkT": np.ascontiguousarray(Wk.astype(np.float32).T).astype(MM_NP),
        "wvT": np.ascontiguousarray(Wv.astype(np.float32).T).astype(MM_NP),
        "woT": np.ascontiguousarray(Wo.astype(np.float32).T).astype(MM_NP),
        "bq": (bq.astype(np.float32) * scale),
        "bo": (bo.astype(np.float32) + Wo.astype(np.float32) @ bv.astype(np.float32)),
    }
    causal = _is_causal(np.asarray(mask))
    if not causal:
        common["maskT"] = np.ascontiguousarray(np.asarray(mask, np.float32).T)
    in_maps = [dict(common, xT=xT[b]) for b in range(B)]
    return causal, in_maps


def kernel(x, mask, Wq, bq, Wk, Wv, bv, Wo, bo):
    causal, in_maps = prep_inputs(x, mask, Wq, bq, Wk, Wv, bv, Wo, bo)
    nc = _get_nc(causal)
    res = run_bass_kernel_spmd(nc, in_maps, list(range(B))).results
    out = np.stack([res[b]["yT"].T for b in range(B)])
    return np.ascontiguousarray(out.astype(np.float32))


# revision 29
# speedup vs baseline: 1.0375x; 1.0051x over previous
"""Multi-head attention (B=8, S=1500, E=1024, H=16, D=64) on 8 trn2 NeuronCores.

Sharding: pure data-parallel over batch — core b computes batch element b
end-to-end (no collectives). Host pre-transposes x and the weights so every
device-side matmul has its contraction dim on the SBUF partition axis, and
folds the 1/sqrt(D) scale into Wq/bq and the V-bias into the output bias
(bo_eff = bo + Wo @ bv), so the device kernel never touches bv.

Device pipeline per core (all f32 storage, matmuls run as float32r):
  QT = (Wq/8)^T-proj of x^T   [1024, 1500]  (f-on-partition; bias bq/8 per-partition)
  KT = Wk^T-proj              [1024, 1500]
  V_aug = x @ Wv^T with a ones-column appended per head  [1500, 16*65]
  per (i-chunk, head): scoresT[j, i] via matmul(lhsT=KT_h, rhs=QT_h);
    exp on ACT (no max-subtraction: |scores| <~ 30, safely inside f32);
    causal masking structurally (affine_select zero-fill on diagonal blocks)
    or via an additive mask tensor (general path);
  out^T + rowsums in ONE matmul: lhsT=[V_h | 1] (65 cols), rhs=attnT;
  normalize: recip of the sums row, rank-1 matmul broadcast across 64
    partitions, multiply on eviction -> AO^T;
  yT = Wo^T-proj of AO^T + bo_eff  -> DRAM [1024, 1500], host transposes back.
"""

import sys
import numpy as np
import ml_dtypes

for _p in ("/opt/trn_rl_repo",):
    if _p not in sys.path:
        sys.path.append(_p)

import concourse.bass as bass
import concourse.mybir as mybir
import concourse.tile as tile
from concourse import bacc
from concourse.bass_utils import run_bass_kernel_spmd

F32 = mybir.dt.float32

B, S, E, H, D = 8, 1500, 1024, 16, 64
P = 128
NEG = -1e9


def _chunks(total, step):
    return [(c0, min(step, total - c0)) for c0 in range(0, total, step)]


def _wslices(dram_ap, col0, cols):
    """[E, E] weight -> [P, E//P, cols] AP for a column slice (k on partition)."""
    return dram_ap.rearrange("(kt p) f -> p kt f", p=P)[:, :, col0:col0 + cols]


def build(causal: bool, mm_dt=mybir.dt.bfloat16):
    KT_N = E // P            # k-tiles over the embedding dim
    FT_N = E // P            # f-tiles
    R_CH = _chunks(S, 512)   # i/r chunks
    JB = _chunks(S, P)       # j blocks
    H_LOC = E // D
    nc = bacc.Bacc("TRN2", target_bir_lowering=False, debug=False, num_devices=8)
    MD = mm_dt  # dtype for every matmul operand chain
    NRM = mybir.dt.float32r if MD == mybir.dt.bfloat16 else MD

    xT = nc.dram_tensor("xT", [E, S], MD, kind="ExternalInput")
    wqT = nc.dram_tensor("wqT", [E, E], MD, kind="ExternalInput")
    wkT = nc.dram_tensor("wkT", [E, E], MD, kind="ExternalInput")
    wvT = nc.dram_tensor("wvT", [E, E], MD, kind="ExternalInput")
    woT = nc.dram_tensor("woT", [E, E], MD, kind="ExternalInput")
    bq = nc.dram_tensor("bq", [E], F32, kind="ExternalInput")
    bo = nc.dram_tensor("bo", [E], F32, kind="ExternalInput")
    maskT = None
    if not causal:
        maskT = nc.dram_tensor("maskT", [S, S], F32, kind="ExternalInput")
    yT = nc.dram_tensor("yT", [E, S], F32, kind="ExternalOutput")

    def mm(ap):
        return ap

    nc._allow_low_precision_reason = "low-precision matmul operand chain"
    with tile.TileContext(nc) as tc:
        with (
            tc.tile_pool(name="persist", bufs=1) as pers,
            tc.tile_pool(name="wqkp", bufs=2) as wqkp,
            tc.tile_pool(name="wvp", bufs=1) as wvp,
            tc.tile_pool(name="wop", bufs=1) as wop,
            tc.tile_pool(name="attn", bufs=3) as apool,
            tc.tile_pool(name="small", bufs=3) as spool,
            tc.tile_pool(name="evp", bufs=3) as evp,
            tc.tile_pool(name="psP", bufs=1, space="PSUM") as psP,
            tc.tile_pool(name="psS", bufs=1, space="PSUM") as psS,
            tc.tile_pool(name="psO", bufs=1, space="PSUM") as psO,
            tc.tile_pool(name="psY", bufs=1, space="PSUM") as psY,
        ):
            ones64 = pers.tile([1, D], NRM, name="ones64")
            nc.vector.memset(ones64[:].bitcast(F32), 1.0)
            bq_sb = pers.tile([P, FT_N], F32, name="bq_sb")
            nc.sync.dma_start(out=bq_sb[:], in_=bq.ap().rearrange("(t p) -> p t", p=P))
            bo_sb = pers.tile([P, FT_N], F32, name="bo_sb")
            nc.sync.dma_start(out=bo_sb[:], in_=bo.ap().rearrange("(t p) -> p t", p=P))

            # upper-triangular (incl diag) 0/1 mask for diagonal attn blocks
            tri32 = pers.tile([P, P], F32, name="tri32")
            nc.gpsimd.memset(tri32[:], 1.0)
            nc.gpsimd.affine_select(
                out=tri32[:], in_=tri32[:],
                pattern=[[1, P]], compare_op=mybir.AluOpType.is_ge,
                fill=0.0, base=0, channel_multiplier=-1,
            )
            tri = pers.tile([P, P], MD, name="tri")
            nc.vector.tensor_copy(out=tri[:], in_=tri32[:])

            XT = [pers.tile([P, S], MD, name=f"xt{kt}") for kt in range(KT_N)]
            QT = [pers.tile([P, S], MD, name=f"qt{ft}") for ft in range(FT_N)]
            KTs = [pers.tile([P, S], MD, name=f"kt{ft}") for ft in range(FT_N)]
            VA = [pers.tile([P, H_LOC * (D + 1)], MD, name=f"va{rt}")
                  for rt in range(len(JB))]
            AOT = [pers.tile([P, S], MD, name=f"aot{ft}") for ft in range(FT_N)]

            # ---- V projection (natural layout, ones column appended) ----
            hpc = 512 // D  # heads per 512-wide f chunk
            fchunks = _chunks(E, 512)
            wv_tiles = [wvp.tile([P, KT_N, 512], MD, name=f"wv{fc}", tag=f"wv{fc}")
                        for fc in range(len(fchunks))]
            # interleave wv-slice and xT-tile loads so the first V matmuls
            # can issue as soon as (wv[:,0,:], xT[0]) land
            for kt in range(KT_N):
                for fc, (f0, fw) in enumerate(fchunks):
                    nc.sync.dma_start(
                        out=wv_tiles[fc][:, kt, :fw],
                        in_=_wslices(wvT.ap(), f0, fw)[:, kt, :])
                nc.sync.dma_start(out=XT[kt][:],
                                  in_=xT[kt * P:(kt + 1) * P, :])

            def emit_v(rts):
                for rt in rts:
                    if rt >= len(JB):
                        continue
                    r0, rsz = JB[rt]
                    for fc, (f0, fw) in enumerate(fchunks):
                        wt = wv_tiles[fc]
                        ps = psP.tile([P, 512], F32, name="pv", tag="pp", bufs=2)
                        for kt in range(KT_N):
                            nc.tensor.matmul(
                                ps[:rsz, :fw],
                                mm(XT[kt][:, r0:r0 + rsz]),
                                mm(wt[:, kt, :fw]),
                                start=(kt == 0), stop=(kt == KT_N - 1),
                            )
                        dst = VA[rt][:].rearrange("p (h c) -> p h c", c=D + 1)
                        nc.vector.tensor_copy(
                            out=dst[:rsz, fc * hpc:fc * hpc + fw // D, 0:D],
                            in_=ps[:rsz, :fw].rearrange("p (h d) -> p h d", d=D),
                        )
                    va3 = VA[rt][:].rearrange("p (h c) -> p h c", c=D + 1)
                    if MD == mybir.dt.float32r:
                        nc.gpsimd.memset(va3[:rsz, :, D:D + 1].bitcast(F32), 1.0)
                    else:
                        nc.gpsimd.memset(va3[:rsz, :, D:D + 1], 1.0)

            def proj_qk_gen(ft):
                for which, wdram, dst in (("q", wqT, QT), ("k", wkT, KTs)):
                    wt = wqkp.tile([P, KT_N, P], MD, name="wqk", tag="wqk")
                    nc.sync.dma_start(out=wt[:], in_=_wslices(wdram.ap(), ft * P, P))
                    for rc, (c0, cw) in enumerate(R_CH):
                        ps = psP.tile([P, 512], F32, name="pp", tag="pp", bufs=2)
                        for kt in range(KT_N):
                            nc.tensor.matmul(
                                ps[:, :cw],
                                mm(wt[:, kt, :]),
                                mm(XT[kt][:, c0:c0 + cw]),
                                start=(kt == 0), stop=(kt == KT_N - 1),
                            )
                        if which == "q":
                            nc.vector.tensor_scalar(
                                out=dst[ft][:, c0:c0 + cw], in0=ps[:, :cw],
                                scalar1=bq_sb[:, ft:ft + 1], scalar2=None,
                                op0=mybir.AluOpType.add,
                            )
                        else:
                            nc.vector.tensor_copy(
                                out=dst[ft][:, c0:c0 + cw], in_=ps[:, :cw])
                        yield

            def proj_qk(ft):
                for _ in proj_qk_gen(ft):
                    pass

            def attn_ft(ic, ft, mtiles, filler=None):
                c0, cw = R_CH[ic]
                nblk = (min(len(JB), (c0 + cw + P - 1) // P)
                        if causal else len(JB))
                pso = [psO.tile([D + 1, 512], F32, name=f"po{half}",
                                tag="po", bufs=2)
                       for half in range(2)]
                # diagonal-containing blocks first so the chunk-end attnV
                # gates on a short (non-masked) exp chain
                if causal:
                    cut = max(0, nblk - (cw + P - 1) // P)
                    order = list(range(cut, nblk)) + list(range(cut))
                else:
                    order = list(range(nblk))
                for n_i, jb in enumerate(order):
                    j0, jsz = JB[jb]
                    vo = max(0, j0 - c0) if causal else 0
                    pss, ats = [], []
                    for half in range(2):
                        d0 = D * half
                        ps = psS.tile([P, 512], F32, name=f"ps{half}",
                                      tag="ps", bufs=3)
                        # adjacent half-pair at row groups (0,0)/(64,0):
                        # runs concurrently in the PE array
                        nc.tensor.matmul(
                            ps[:jsz, vo:cw],
                            mm(KTs[ft][d0:d0 + D, j0:j0 + jsz]),
                            mm(QT[ft][d0:d0 + D, c0 + vo:c0 + cw]),
                            start=True, stop=True,
                            tile_position=(d0, 0),
                        )
                        pss.append(ps)
                    for half in range(2):
                        ps = pss[half]
                        if not causal:
                            nc.vector.tensor_tensor(
                                out=ps[:jsz, :cw], in0=ps[:jsz, :cw],
                                in1=mtiles[jb][:jsz, :cw],
                                op=mybir.AluOpType.add,
                            )
                        at = apool.tile([P, 512], MD, name=f"at{half}")
                        nc.scalar.activation(
                            out=at[:jsz, vo:cw], in_=ps[:jsz, vo:cw],
                            func=mybir.ActivationFunctionType.Exp,
                        )
                        if causal and j0 >= c0:
                            # zero attn where j > i on the diagonal square
                            nc.vector.tensor_tensor(
                                out=at[:jsz, vo:vo + jsz],
                                in0=at[:jsz, vo:vo + jsz],
                                in1=tri[:jsz, :jsz],
                                op=mybir.AluOpType.mult,
                            )
                        ats.append(at)
                    va3 = VA[jb][:].rearrange("p (h c) -> p h c", c=D + 1)
                    for half in range(2):
                        nc.tensor.matmul(
                            pso[half][:, vo:cw],
                            mm(va3[:jsz, 2 * ft + half, :]),
                            mm(ats[half][:jsz, vo:cw]),
                            start=(n_i == 0), stop=(n_i == nblk - 1),
                        )
                    if filler is not None and n_i % 4 == 3:
                        filler()
                ssums = []
                for half in range(2):
                    ssum = spool.tile([1, 512], NRM, name=f"ssum{half}",
                                      tag="ssum")
                    nc.vector.tensor_copy(
                        out=ssum[:, :cw], in_=pso[half][D:D + 1, :cw])
                    ssums.append(ssum)
                for half in range(2):
                    d0 = D * half
                    psb = psY.tile([D, 512], F32, name="psb", tag="pyb", bufs=1)
                    nc.tensor.matmul(
                        psb[:, :cw], mm(ones64[:, :]), mm(ssums[half][:, :cw]),
                        start=True, stop=True,
                    )
                    rb = spool.tile([D, 512], F32, name="rb")
                    nc.vector.reciprocal_approx_fast(
                        out=rb[:, :cw], in_=psb[:, :cw])
                    nc.vector.tensor_tensor(
                        out=AOT[ft][d0:d0 + D, c0:c0 + cw],
                        in0=pso[half][0:D, :cw], in1=rb[:, :cw],
                        op=mybir.AluOpType.mult,
                    )

            def emit_yt(ot, rc, wo_t):
                c0, cw = R_CH[rc]
                psy = psP.tile([P, 512], F32, name="py", tag="pp", bufs=2)
                for ft in range(FT_N):
                    nc.tensor.matmul(
                        psy[:, :cw],
                        mm(wo_t[:, ft, :]),
                        mm(AOT[ft][:, c0:c0 + cw]),
                        start=(ft == 0), stop=(ft == FT_N - 1),
                    )
                yt = evp.tile([P, 512], F32, name="yt", tag="yt")
                nc.vector.tensor_scalar(
                    out=yt[:, :cw], in0=psy[:, :cw],
                    scalar1=bo_sb[:, ot:ot + 1], scalar2=None,
                    op0=mybir.AluOpType.add,
                )
                nc.sync.dma_start(
                    out=yT[ot * P:(ot + 1) * P, c0:c0 + cw], in_=yt[:, :cw])

            if causal:
                nb0 = min(len(JB), (R_CH[0][0] + R_CH[0][1] + P - 1) // P)
                emit_v(range(nb0))
                proj_qk(0)
                nbp = nb0
                for ft in range(FT_N):
                    gen = proj_qk_gen(ft + 1) if ft + 1 < FT_N else None

                    def pump():
                        if gen is not None:
                            next(gen, None)

                    for ic in range(len(R_CH)):
                        attn_ft(ic, ft, None, filler=pump)
                        if ft == 0 and ic + 1 < len(R_CH):
                            c0n, cwn = R_CH[ic + 1]
                            nbn = min(len(JB), (c0n + cwn + P - 1) // P)
                            emit_v(range(nbp, nbn))
                            nbp = nbn
                    if gen is not None:
                        for _ in gen:
                            pass
                for ot in range(FT_N):
                    wt = wop.tile([P, KT_N, P], MD, name=f"wo{ot}", tag="wo",
                                  bufs=2)
                    nc.sync.dma_start(out=wt[:], in_=_wslices(woT.ap(), ot * P, P))
                    for rc in range(len(R_CH)):
                        emit_yt(ot, rc, wt)
            else:
                emit_v(range(len(JB)))
                for ft in range(FT_N):
                    proj_qk(ft)
                with tc.tile_pool(name="maskp", bufs=1) as mpool:
                    for ic, (c0, cw) in enumerate(R_CH):
                        mtiles = []
                        for jb, (j0, jsz) in enumerate(JB):
                            mt = mpool.tile([P, 512], F32, name=f"m{jb}")
                            nc.sync.dma_start(
                                out=mt[:jsz, :cw],
                                in_=maskT[j0:j0 + jsz, c0:c0 + cw])
                            mtiles.append(mt)
                        for ft in range(FT_N):
                            attn_ft(ic, ft, mtiles)
                for ot in range(FT_N):
                    wt = wop.tile([P, KT_N, P], MD, name=f"wo{ot}", tag="wo",
                                  bufs=2)
                    nc.sync.dma_start(out=wt[:], in_=_wslices(woT.ap(), ot * P, P))
                    for rc in range(len(R_CH)):
                        emit_yt(ot, rc, wt)

    nc.compile()
    return nc


_CACHE: dict = {}


def _get_nc(causal: bool):
    if causal not in _CACHE:
        _CACHE[causal] = build(causal)
    return _CACHE[causal]


def _is_causal(mask: np.ndarray) -> bool:
    if mask.shape != (S, S):
        return False
    expect = np.where(np.tril(np.ones((S, S), dtype=bool)), np.float32(0.0),
                      np.float32(NEG))
    return bool(np.array_equal(mask, expect))


MM_NP = ml_dtypes.bfloat16  # numpy dtype matching build()'s default mm_dt


def prep_inputs(x, mask, Wq, bq, Wk, Wv, bv, Wo, bo):
    """Host-side preprocessing shared by kernel() and the bench harness."""
    scale = np.float32(1.0 / np.sqrt(D))
    xT = np.ascontiguousarray(np.transpose(x, (0, 2, 1)).astype(np.float32)).astype(MM_NP)
    common = {
        "wqT": np.ascontiguousarray((Wq.astype(np.float32) * scale).T).astype(MM_NP),
        "wkT": np.ascontiguousarray(Wk.astype(np.float32).T).astype(MM_NP),
        "wvT": np.ascontiguousarray(Wv.astype(np.float32).T).astype(MM_NP),
        "woT": np.ascontiguousarray(Wo.astype(np.float32).T).astype(MM_NP),
        "bq": (bq.astype(np.float32) * scale),
        "bo": (bo.astype(np.float32) + Wo.astype(np.float32) @ bv.astype(np.float32)),
    }
    causal = _is_causal(np.asarray(mask))
    if not causal:
        common["maskT"] = np.ascontiguousarray(np.asarray(mask, np.float32).T)
    in_maps = [dict(common, xT=xT[b]) for b in range(B)]
    return causal, in_maps


def kernel(x, mask, Wq, bq, Wk, Wv, bv, Wo, bo):
    causal, in_maps = prep_inputs(x, mask, Wq, bq, Wk, Wv, bv, Wo, bo)
    nc = _get_nc(causal)
    res = run_bass_kernel_spmd(nc, in_maps, list(range(B))).results
    out = np.stack([res[b]["yT"].T for b in range(B)])
    return np.ascontiguousarray(out.astype(np.float32))
